# revision 11
# baseline (speedup 1.0000x reference)
"""3-layer GAT message passing on 8 Trainium2 NeuronCores (Bass/Tile).

Sharding: nodes split into 8 contiguous dst-ranges with ~equal edge counts,
then into fixed 128-node windows: window g of core c = table rows
c*OWNP + g*128 .. +128, so one SPMD program serves all cores.

Per layer, each node's 256-byte table row is [fp8 h*16 (cout) | fp8 1.0 |
pad | fp16 asrc | fp16 adst | pad] (layer 2: fp16 h, 'one' at the same byte
offset 128). Phase A computes rows from h @ [W | W@a_src | W@a_dst] and
AllGathers them. Phase B dma-gathers rows by edge src (A/B table halves for
the int16 index range), derives the edge weight
wv = exp(min(lrelu(asrc+adst), 18) - 10) (a fixed shift is exact: any
per-window constant cancels in num/den), folds wv into the one-hot scatter
matrix S', and one matmul per 128-edge subtile accumulates
[sum wv*h | sum wv] in PSUM: out = num/den + bias. Layer 2 fuses
global_add_pool (one-hot batch matmul into persistent PSUM) + AllReduce.
"""

import dataclasses
import math

import numpy as np


@dataclasses.dataclass
class Cfg:
    n_nodes: int = 50000
    n_edges: int = 800000
    in_c: int = 128
    hid_c: int = 128
    out_c: int = 64
    n_graphs: int = 64
    neg_slope: float = 0.2
    ncores: int = 8
    win: int = 128           # dst window (128 nodes per group)
    smax: int = 40           # max subtiles per chunk per region
    zshift: float = 10.0     # fixed softmax stabilizer shift
    zclamp: float = 18.0     # logit clamp (garbage-slot safety)
    h8scale: float = 16.0    # fp8 table quantization scale
    single_packet: bool = False


FULL = Cfg()


# ----------------------------------------------------------------------------
# host-side planning (pure numpy)
# ----------------------------------------------------------------------------

def build_plan(edge_index, batch, cfg: Cfg):
    N = cfg.n_nodes
    W = cfg.win
    src0 = np.concatenate([edge_index[0].astype(np.int64), np.arange(N)])
    dst0 = np.concatenate([edge_index[1].astype(np.int64), np.arange(N)])
    order = np.argsort(dst0, kind="stable")
    src_g = src0[order]
    dst_g = dst0[order]
    E = src_g.shape[0]

    deg = np.bincount(dst_g, minlength=N)
    cume = np.cumsum(deg)
    bounds = [0]
    for c in range(1, cfg.ncores):
        bounds.append(int(np.searchsorted(cume, c * E / cfg.ncores)))
    bounds.append(N)

    nnode_c = [bounds[c + 1] - bounds[c] for c in range(cfg.ncores)]
    gtot = max(math.ceil(n / W) for n in nnode_c)
    ownp = gtot * W
    bbase = (cfg.ncores // 2) * ownp
    assert bbase <= 32767, f"table half too large: ownp={ownp}"

    # A-half of the table = cores 0..3 == src < bounds[4]
    srcA = src_g < bounds[cfg.ncores // 2]
    degA = np.bincount(dst_g, weights=srcA.astype(np.float64), minlength=N).astype(np.int64)
    degB = deg - degA

    remap = np.full(N, -1, np.int64)
    for c in range(cfg.ncores):
        n = nnode_c[c]
        remap[bounds[c]:bounds[c + 1]] = c * ownp + np.arange(n)
    src_n = remap[src_g]

    # per-(core, window) A/B edge counts -> global per-window subtile counts
    sA_g = np.ones(gtot, np.int64)
    sB_g = np.ones(gtot, np.int64)
    for c in range(cfg.ncores):
        for g in range(gtot):
            n0 = bounds[c] + g * W
            n1 = min(bounds[c] + (g + 1) * W, bounds[c + 1])
            if n0 >= n1:
                continue
            eA = int(degA[n0:n1].sum())
            eB = int(degB[n0:n1].sum())
            sA_g[g] = max(sA_g[g], math.ceil(max(eA, 1) / 128))
            sB_g[g] = max(sB_g[g], math.ceil(max(eB, 1) / 128))
    assert sA_g.max() <= cfg.smax and sB_g.max() <= cfg.smax

    # global chunk schedule: consecutive windows while sums fit
    chunks = []
    cur, ca, cb = [], 0, 0
    for g in range(gtot):
        if cur and (ca + sA_g[g] > cfg.smax or cb + sB_g[g] > cfg.smax):
            chunks.append(cur)
            cur, ca, cb = [], 0, 0
        cur.append(g)
        ca += sA_g[g]
        cb += sB_g[g]
    if cur:
        chunks.append(cur)

    sched = []
    offA = offB = offD = 0
    for ck in chunks:
        sAtot = int(sum(sA_g[g] for g in ck))
        sBtot = int(sum(sB_g[g] for g in ck))
        wins = []
        pa, pb = 0, 0
        for g in ck:
            wins.append({"g": int(g), "sA": int(sA_g[g]), "sB": int(sB_g[g]),
                         "colA": pa, "colB": sAtot + pb})
            pa += int(sA_g[g])
            pb += int(sB_g[g])
        sched.append({"sA": sAtot, "sB": sBtot, "offA": offA, "offB": offB,
                      "offD": offD, "wins": wins})
        offA += sAtot * 128 // 16
        offB += sBtot * 128 // 16
        offD += sAtot + sBtot
    totA16, totB16, totD = offA, offB, offD

    # per-core gather index / dstl arrays in the global layout
    plans = []
    for c in range(cfg.ncores):
        lo = np.searchsorted(dst_g, bounds[c])
        hi = np.searchsorted(dst_g, bounds[c + 1])
        s = src_n[lo:hi]
        d = dst_g[lo:hi]
        sa = srcA[order][lo:hi] if False else (s < bbase)
        nstart = lo + np.searchsorted(d, np.arange(bounds[c], bounds[c + 1] + 1))

        idxA = np.zeros((128, totA16), np.int16)
        idxB = np.zeros((128, totB16), np.int16)
        dstl = np.full((128, totD), -1.0, np.float16)
        for ck in sched:
            nA, nB = ck["sA"] * 128, ck["sB"] * 128
            arrA = np.zeros(nA, np.int64)
            arrB = np.zeros(nB, np.int64)
            dl = np.full((128, ck["sA"] + ck["sB"]), -1.0, np.float16)
            for w in ck["wins"]:
                n0 = bounds[c] + w["g"] * W
                n1 = min(bounds[c] + (w["g"] + 1) * W, bounds[c + 1])
                if n0 >= n1:
                    continue
                e0 = nstart[n0 - bounds[c]] - lo
                e1 = nstart[n1 - bounds[c]] - lo
                sl = s[e0:e1]
                dloc = (d[e0:e1] - n0).astype(np.int64)
                m = sa[e0:e1]
                iA, dA = sl[m], dloc[m]
                iB, dB = sl[~m] - bbase, dloc[~m]
                assert len(iA) <= w["sA"] * 128 and len(iB) <= w["sB"] * 128
                arrA[w["colA"] * 128: w["colA"] * 128 + len(iA)] = iA
                arrB[(w["colB"] - ck["sA"]) * 128:
                     (w["colB"] - ck["sA"]) * 128 + len(iB)] = iB
                ii = np.arange(len(iA))
                dl[ii % 128, w["colA"] + ii // 128] = dA
                ii = np.arange(len(iB))
                dl[ii % 128, w["colB"] + ii // 128] = dB
            for arr, n, out, off in ((arrA, nA, idxA, ck["offA"]),
                                     (arrB, nB, idxB, ck["offB"])):
                t = np.zeros((16, n // 16), np.int16)
                t[np.arange(n) % 16, np.arange(n) // 16] = arr.astype(np.int16)
                out[:, off:off + n // 16] = np.tile(t, (8, 1))
            dstl[:, ck["offD"]:ck["offD"] + ck["sA"] + ck["sB"]] = dl

        batchloc = np.full((W, gtot), -1.0, np.float16)
        for g in range(gtot):
            n0 = bounds[c] + g * W
            n1 = min(bounds[c] + (g + 1) * W, bounds[c + 1])
            if n0 < n1:
                batchloc[:n1 - n0, g] = batch[n0:n1].astype(np.float16)

        plans.append({"idxA": idxA, "idxB": idxB, "dstl": dstl,
                      "batchloc": batchloc, "bounds": (bounds[c], bounds[c + 1])})

    meta = {"gtot": gtot, "ownp": ownp, "bbase": bbase, "sched": sched,
            "totA16": totA16, "totB16": totB16, "totD": totD}
    return plans, meta


# ----------------------------------------------------------------------------
# device program (shared across all 8 cores)
# ----------------------------------------------------------------------------

def build_nc(cfg: Cfg, meta, debug=False):
    import concourse.bass as bass  # noqa: F401
    import concourse.mybir as mybir
    import concourse.tile as tile
    from concourse import bacc

    fp16 = mybir.dt.float16
    fp8 = mybir.dt.float8e4
    f32 = mybir.dt.float32
    i16 = mybir.dt.int16
    u8 = mybir.dt.uint8
    AL = mybir.AluOpType
    AF = mybir.ActivationFunctionType
    AX = mybir.AxisListType

    gtot, OWNP, bbase = meta["gtot"], meta["ownp"], meta["bbase"]
    sched = meta["sched"]
    W = cfg.win
    couts = [cfg.hid_c, cfg.hid_c, cfg.out_c]
    cins = [cfg.in_c, cfg.hid_c, cfg.hid_c]
    rg = [list(range(cfg.ncores))]
    TROWS = cfg.ncores * OWNP
    ntiles = OWNP // 128

    nc = bacc.Bacc("TRN2", target_bir_lowering=False, debug=debug)

    xT = nc.dram_tensor("xT", [cfg.in_c, OWNP], fp16, kind="ExternalInput")
    Wc, Bb = [], []
    for l in range(3):
        Wc.append(nc.dram_tensor(f"wcat{l}", [cins[l], couts[l] + 2], fp16,
                                 kind="ExternalInput"))
        Bb.append(nc.dram_tensor(f"biasbc{l}", [W, couts[l]], f32,
                                 kind="ExternalInput"))
    iota_d = nc.dram_tensor("iota", [128, W], fp16, kind="ExternalInput")
    onesr_d = nc.dram_tensor("onesrow", [1, 128], fp16, kind="ExternalInput")
    idxA_d = nc.dram_tensor("idxA", [128, meta["totA16"]], i16, kind="ExternalInput")
    idxB_d = nc.dram_tensor("idxB", [128, meta["totB16"]], i16, kind="ExternalInput")
    dstl_d = nc.dram_tensor("dstl", [128, meta["totD"]], fp16, kind="ExternalInput")
    bloc_d = nc.dram_tensor("batchloc", [W, gtot], fp16, kind="ExternalInput")
    out_ext = nc.dram_tensor("out", [cfg.n_graphs, cfg.out_c], f32,
                             kind="ExternalOutput")

    ownt, tbl, adst_d = [], [], []
    for l in range(3):
        ownt.append(nc.dram_tensor(f"ownt{l}", [OWNP, 256], u8))
        tbl.append(nc.dram_tensor(f"tbl{l}", [TROWS, 256], u8, addr_space="Shared"))
        adst_d.append(nc.dram_tensor(f"adst{l}", [OWNP, 1], fp16))
    hb = {l: nc.dram_tensor(f"hb{l}", [OWNP, cfg.hid_c], fp16) for l in (1, 2)}
    pool_l = nc.dram_tensor("pool_local", [cfg.n_graphs, cfg.out_c], f32)
    pool_s = nc.dram_tensor("pool_shared", [cfg.n_graphs, cfg.out_c], f32,
                            addr_space="Shared")

    import contextlib
    with tile.TileContext(nc) as tc, contextlib.ExitStack() as ctx:
        cpool = ctx.enter_context(tc.tile_pool(name="consts", bufs=1))
        apool = ctx.enter_context(tc.tile_pool(name="phasea", bufs=3))
        gpool = ctx.enter_context(tc.tile_pool(name="gather", bufs=2))
        wpool = ctx.enter_context(tc.tile_pool(name="work", bufs=3))
        spool = ctx.enter_context(tc.tile_pool(name="sprime", bufs=2))
        pspool = ctx.enter_context(tc.tile_pool(name="ps", bufs=2, space="PSUM"))
        psw = ctx.enter_context(tc.tile_pool(name="psw", bufs=3, space="PSUM"))
        psacc = ctx.enter_context(tc.tile_pool(name="psacc", bufs=1, space="PSUM"))

        from concourse import library_config
        nc.gpsimd.load_library(library_config.mlp)

        # ---- constants ----
        iota = cpool.tile([128, W], fp16, tag="iota")
        nc.sync.dma_start(iota[:], iota_d[:, :])
        onesr = cpool.tile([1, 128], fp16, tag="onesr")
        nc.sync.dma_start(onesr[:], onesr_d[:, :])
        wcat_sb, bias_sb = [], []
        for l in range(3):
            t = cpool.tile([cins[l], couts[l] + 2], fp16, tag=f"wc{l}")
            nc.sync.dma_start(t[:], Wc[l][:, :])
            wcat_sb.append(t)
            t = cpool.tile([W, couts[l]], f32, tag=f"bb{l}")
            nc.sync.dma_start(t[:], Bb[l][:, :])
            bias_sb.append(t)
        bloc_sb = cpool.tile([W, gtot], fp16, tag="bloc")
        nc.sync.dma_start(bloc_sb[:], bloc_d[:, :])

        zero1 = cpool.tile([128, 1], f32, tag="zero1")
        nc.vector.memset(zero1[:], 0.0)

        psum_pool_acc = psacc.tile([cfg.n_graphs, cfg.out_c], f32, tag="pool")

        for l in range(3):
            cout, cin = couts[l], cins[l]
            use8 = l < 2
            # ======== phase A: own table shard ========
            for t in range(ntiles):
                lhsT = apool.tile([cin, 128], fp16, tag="lhsT")
                if l == 0:
                    nc.sync.dma_start(lhsT[:], xT[:, t * 128:(t + 1) * 128])
                else:
                    nc.sync.dma_start_transpose(
                        lhsT[:], hb[l][t * 128:(t + 1) * 128, 0:cin])
                psA = pspool.tile([128, cout + 2], f32, tag="pa", space="PSUM")
                nc.tensor.matmul(out=psA[:], lhsT=lhsT[:], rhs=wcat_sb[l][:],
                                 start=True, stop=True)
                row = apool.tile([128, 256], u8, tag="row")
                if use8:
                    nc.scalar.activation(out=row[:, 0:128].bitcast(fp8),
                                         in_=psA[:, 0:cout], func=AF.Copy,
                                         scale=cfg.h8scale)
                    nc.vector.memset(row[:, 128:130].bitcast(fp8), 1.0)
                else:
                    nc.scalar.activation(out=row[:, 0:128].bitcast(fp16)[:, 0:cout],
                                         in_=psA[:, 0:cout], func=AF.Copy)
                    nc.vector.memset(row[:, 128:130].bitcast(fp16), 1.0)
                nc.vector.tensor_copy(out=row[:, 130:134].bitcast(fp16),
                                      in_=psA[:, cout:cout + 2])
                nc.sync.dma_start(ownt[l][t * 128:(t + 1) * 128, :], row[:])
                adt = apool.tile([128, 1], fp16, tag="adt")
                nc.vector.tensor_copy(out=adt[:], in_=psA[:, cout + 1:cout + 2])
                nc.sync.dma_start(adst_d[l][t * 128:(t + 1) * 128, :], adt[:])

            # ======== allgather ========
            nc.gpsimd.collective_compute(
                "AllGather", AL.bypass, replica_groups=rg,
                ins=[ownt[l][:, :]], outs=[tbl[l][:, :]])

            # ======== phase B: edge processing ========
            for ck in sched:
                sA, sB = ck["sA"], ck["sB"]
                nA, nB = sA * 128, sB * 128
                ixa = gpool.tile([128, sA * 8], i16, tag="ixa")
                nc.sync.dma_start(ixa[:], idxA_d[:, ck["offA"]:ck["offA"] + sA * 8])
                ixb = gpool.tile([128, sB * 8], i16, tag="ixb")
                nc.sync.dma_start(ixb[:], idxB_d[:, ck["offB"]:ck["offB"] + sB * 8])
                dsl = gpool.tile([128, sA + sB], fp16, tag="dsl")
                nc.sync.dma_start(dsl[:], dstl_d[:, ck["offD"]:ck["offD"] + sA + sB])
                gA = gpool.tile([128, sA, 256], u8, tag="gA")
                nc.gpsimd.dma_gather(gA[:], tbl[l][:, :], ixa[:], nA, nA, 256,
                                     single_packet=cfg.single_packet)
                gB = gpool.tile([128, sB, 256], u8, tag="gB")
                nc.gpsimd.dma_gather(gB[:], tbl[l][bbase:, :], ixb[:], nB, nB, 256,
                                     single_packet=cfg.single_packet)

                # adst of the chunk's own window nodes, broadcast to all
                # 128 partitions via K=1 matmuls (<=512 psum cols per mm)
                nwin = len(ck["wins"])
                g0 = ck["wins"][0]["g"]
                nwcols = nwin * W
                awr = wpool.tile([1, nwcols], fp16, tag="awr")
                nc.sync.dma_start(
                    awr[:], adst_d[l][g0 * W:g0 * W + nwcols, :]
                    .rearrange("n x -> x n"))
                awbc = wpool.tile([128, nwcols], fp16, tag="awbc")
                for c0 in range(0, nwcols, 512):
                    c1 = min(c0 + 512, nwcols)
                    psaw = pspool.tile([128, c1 - c0], f32, tag="psaw",
                                       space="PSUM")
                    nc.tensor.matmul(out=psaw[:], lhsT=onesr[:],
                                     rhs=awr[:, c0:c1], start=True, stop=True)
                    nc.vector.tensor_copy(out=awbc[:, c0:c1], in_=psaw[:])

                for iw, w in enumerate(ck["wins"]):
                    g = w["g"]
                    nsub = w["sA"] + w["sB"]
                    # S = (iota == dstl) one-hot, per region
                    sp = spool.tile([128, nsub, W], fp16, tag="sp")
                    for (c0, nS, o0) in ((w["colA"], w["sA"], 0),
                                         (w["colB"], w["sB"], w["sA"])):
                        nc.vector.tensor_tensor(
                            out=sp[:, o0:o0 + nS, :],
                            in0=iota[:][:, None, :].to_broadcast([128, nS, W]),
                            in1=dsl[:, c0:c0 + nS][:, :, None].to_broadcast(
                                [128, nS, W]),
                            op=AL.is_equal)
                    # adst per slot = sum_w S * adst_win
                    tmpa = spool.tile([128, nsub, W], fp16, tag="tmpa")
                    nc.vector.tensor_tensor(
                        out=tmpa[:],
                        in0=sp[:],
                        in1=awbc[:, iw * W:(iw + 1) * W][:, None, :]
                        .to_broadcast([128, nsub, W]),
                        op=AL.mult)
                    z = wpool.tile([128, nsub], f32, tag="z")
                    nc.vector.tensor_reduce(out=z[:], in_=tmpa[:],
                                            axis=AX.X, op=AL.add)
                    # z += asrc (from gathered rows, fp16 elem 65)
                    for (gbuf, jj0, nS, o0) in (
                            (gA, w["colA"], w["sA"], 0),
                            (gB, w["colB"] - sA, w["sB"], w["sA"])):
                        nc.vector.tensor_tensor(
                            out=z[:, o0:o0 + nS, None],
                            in0=z[:, o0:o0 + nS, None],
                            in1=gbuf[:].bitcast(fp16)[:, jj0:jj0 + nS, 65:66],
                            op=AL.add)
                    zl = wpool.tile([128, nsub], f32, tag="zl")
                    nc.vector.scalar_tensor_tensor(
                        out=zl[:], in0=z[:], scalar=cfg.neg_slope, in1=z[:],
                        op0=AL.mult, op1=AL.max)
                    zc = wpool.tile([128, nsub], f32, tag="zc")
                    nc.vector.tensor_scalar(out=zc[:], in0=zl[:],
                                            scalar1=cfg.zclamp,
                                            scalar2=-cfg.zshift,
                                            op0=AL.min, op1=AL.add)
                    wv = wpool.tile([128, nsub], fp16, tag="wv")
                    nc.scalar.activation(out=wv[:], in_=zc[:], func=AF.Exp,
                                         bias=zero1[:], scale=1.0)
                    # S' = S * wv
                    nc.vector.tensor_tensor(
                        out=sp[:], in0=sp[:],
                        in1=wv[:][:, :, None].to_broadcast([128, nsub, W]),
                        op=AL.mult)
                    # scatter matmuls: psO[w, :] += S'[:,j,:].T @ [h|1]
                    psO = psw.tile([W, cout + 1], f32, tag="psO", space="PSUM")
                    nsub = w["sA"] + w["sB"]
                    for j in range(nsub):
                        gbuf, jj = (gA, w["colA"] + j) if j < w["sA"] else \
                                   (gB, w["colB"] - sA + (j - w["sA"]))
                        if use8:
                            rhs = gbuf[:].bitcast(fp8)[:, jj, 0:cout + 1]
                        else:
                            rhs = gbuf[:].bitcast(fp16)[:, jj, 0:cout + 1]
                        nc.tensor.matmul(out=psO[:], lhsT=sp[:, j, :], rhs=rhs,
                                         start=(j == 0), stop=(j == nsub - 1))
                    # out = num/den + bias
                    den = wpool.tile([W, 1], f32, tag="den")
                    scl = cfg.h8scale if use8 else 1.0
                    nc.vector.tensor_scalar(out=den[:], in0=psO[:, cout:cout + 1],
                                            scalar1=scl, scalar2=1e-30,
                                            op0=AL.mult, op1=AL.add)
                    rec = wpool.tile([W, 1], f32, tag="rec")
                    nc.vector.reciprocal(out=rec[:], in_=den[:])
                    orow = wpool.tile([W, cout], f32, tag="orow")
                    nc.vector.scalar_tensor_tensor(
                        out=orow[:], in0=psO[:, 0:cout], scalar=rec[:],
                        in1=bias_sb[l][:], op0=AL.mult, op1=AL.add)
                    oh = wpool.tile([W, cout], fp16, tag="oh")
                    if l < 2:
                        nc.scalar.activation(out=oh[:], in_=orow[:], func=AF.Relu)
                        nc.sync.dma_start(hb[l + 1][g * W:(g + 1) * W, 0:cout],
                                          oh[:])
                    else:
                        nc.vector.tensor_copy(out=oh[:], in_=orow[:])
                        bsel = wpool.tile([W, cfg.n_graphs], fp16, tag="bsel")
                        nc.vector.tensor_tensor(
                            out=bsel[:], in0=iota[0:W, 0:cfg.n_graphs],
                            in1=bloc_sb[:, g:g + 1].to_broadcast(
                                [W, cfg.n_graphs]),
                            op=AL.is_equal)
                        nc.tensor.matmul(out=psum_pool_acc[:], lhsT=bsel[:],
                                         rhs=oh[:], start=(g == 0),
                                         stop=(g == gtot - 1))

        # ---- pool -> allreduce -> out ----
        pooled = cpool.tile([cfg.n_graphs, cfg.out_c], f32, tag="pooled")
        nc.vector.tensor_copy(out=pooled[:], in_=psum_pool_acc[:])
        nc.sync.dma_start(pool_l[:, :], pooled[:])
        nc.gpsimd.collective_compute(
            "AllReduce", AL.add, replica_groups=rg,
            ins=[pool_l[:, :]], outs=[pool_s[:, :]])
        nc.sync.dma_start(out_ext[:, :], pool_s[:, :])

    nc.compile()
    return nc


# ----------------------------------------------------------------------------
# host wrapper
# ----------------------------------------------------------------------------

def make_inputs(inputs, plans, meta, cfg: Cfg):
    x = np.asarray(inputs["x"], np.float32)
    ownp, gtot = meta["ownp"], meta["gtot"]
    iota = np.tile(np.arange(cfg.win, dtype=np.float16), (128, 1))
    wcats, biasbcs = [], []
    for l in range(3):
        Wl = np.asarray(inputs[f"W{l}"], np.float32)
        asl = np.asarray(inputs[f"as{l}"], np.float32)
        adl = np.asarray(inputs[f"ad{l}"], np.float32)
        bl = np.asarray(inputs[f"b{l}"], np.float32)
        wcats.append(np.concatenate(
            [Wl, (Wl @ asl)[:, None], (Wl @ adl)[:, None]],
            axis=1).astype(np.float16))
        biasbcs.append(np.tile(bl[None, :], (cfg.win, 1)).astype(np.float32))
    in_maps = []
    for c in range(cfg.ncores):
        p = plans[c]
        lo, hi = p["bounds"]
        xpad = np.zeros((ownp, cfg.in_c), np.float32)
        xpad[:hi - lo] = x[lo:hi]
        m = {"xT": np.ascontiguousarray(xpad.T).astype(np.float16),
             "iota": iota, "onesrow": np.ones((1, 128), np.float16),
             "idxA": p["idxA"], "idxB": p["idxB"], "dstl": p["dstl"],
             "batchloc": p["batchloc"]}
        for l in range(3):
            m[f"wcat{l}"] = wcats[l]
            m[f"biasbc{l}"] = biasbcs[l]
        in_maps.append(m)
    return in_maps


def kernel(**inputs) -> np.ndarray:
    cfg = FULL
    edge_index = np.asarray(inputs["edge_index"])
    batch = np.asarray(inputs["batch"])
    plans, meta = build_plan(edge_index, batch, cfg)
    in_maps = make_inputs(inputs, plans, meta, cfg)
    nc = build_nc(cfg, meta, debug=False)
    from concourse import bass_utils
    res = bass_utils.run_bass_kernel_spmd(nc, in_maps, core_ids=list(range(cfg.ncores)))
    return np.asarray(res.results[0]["out"], np.float32)


# revision 13
# speedup vs baseline: 1.1765x; 1.1765x over previous
"""3-layer GAT message passing on 8 Trainium2 NeuronCores (Bass/Tile).

Sharding: nodes split into 8 contiguous dst-ranges with ~equal edge counts,
then into fixed 128-node windows: window g of core c = table rows
c*OWNP + g*128 .. +128, so one SPMD program serves all cores.

Per layer, each node's 256-byte table row is [fp8 h*16 (cout) | fp8 1.0 |
pad | fp16 asrc | fp16 adst | pad] (layer 2: fp16 h, 'one' at the same byte
offset 128). Phase A computes rows from h @ [W | W@a_src | W@a_dst] and
AllGathers them. Phase B dma-gathers rows by edge src (A/B table halves for
the int16 index range), derives the edge weight
wv = exp(min(lrelu(asrc+adst), 18) - 10) (a fixed shift is exact: any
per-window constant cancels in num/den), folds wv into the one-hot scatter
matrix S', and one matmul per 128-edge subtile accumulates
[sum wv*h | sum wv] in PSUM: out = num/den + bias. Layer 2 fuses
global_add_pool (one-hot batch matmul into persistent PSUM) + AllReduce.
"""

import dataclasses
import math

import numpy as np


@dataclasses.dataclass
class Cfg:
    n_nodes: int = 50000
    n_edges: int = 800000
    in_c: int = 128
    hid_c: int = 128
    out_c: int = 64
    n_graphs: int = 64
    neg_slope: float = 0.2
    ncores: int = 8
    win: int = 64            # dst window (64 nodes per group)
    smax: int = 40           # max subtiles per chunk per region
    zshift: float = 10.0     # fixed softmax stabilizer shift
    zclamp: float = 18.0     # logit clamp (garbage-slot safety)
    h8scale: float = 16.0    # fp8 table quantization scale
    single_packet: bool = False


FULL = Cfg()


# ----------------------------------------------------------------------------
# host-side planning (pure numpy)
# ----------------------------------------------------------------------------

def build_plan(edge_index, batch, cfg: Cfg):
    N = cfg.n_nodes
    W = cfg.win
    src0 = np.concatenate([edge_index[0].astype(np.int64), np.arange(N)])
    dst0 = np.concatenate([edge_index[1].astype(np.int64), np.arange(N)])
    order = np.argsort(dst0, kind="stable")
    src_g = src0[order]
    dst_g = dst0[order]
    E = src_g.shape[0]

    deg = np.bincount(dst_g, minlength=N)
    cume = np.cumsum(deg)
    bounds = [0]
    for c in range(1, cfg.ncores):
        bounds.append(int(np.searchsorted(cume, c * E / cfg.ncores)))
    bounds.append(N)

    nnode_c = [bounds[c + 1] - bounds[c] for c in range(cfg.ncores)]
    gtot = max(math.ceil(n / W) for n in nnode_c)
    ownp = gtot * W
    bbase = (cfg.ncores // 2) * ownp
    assert bbase <= 32767, f"table half too large: ownp={ownp}"

    # A-half of the table = cores 0..3 == src < bounds[4]
    srcA = src_g < bounds[cfg.ncores // 2]
    degA = np.bincount(dst_g, weights=srcA.astype(np.float64), minlength=N).astype(np.int64)
    degB = deg - degA

    remap = np.full(N, -1, np.int64)
    for c in range(cfg.ncores):
        n = nnode_c[c]
        remap[bounds[c]:bounds[c + 1]] = c * ownp + np.arange(n)
    src_n = remap[src_g]

    # per-(core, window) A/B edge counts -> global per-window subtile counts
    sA_g = np.ones(gtot, np.int64)
    sB_g = np.ones(gtot, np.int64)
    for c in range(cfg.ncores):
        for g in range(gtot):
            n0 = bounds[c] + g * W
            n1 = min(bounds[c] + (g + 1) * W, bounds[c + 1])
            if n0 >= n1:
                continue
            eA = int(degA[n0:n1].sum())
            eB = int(degB[n0:n1].sum())
            sA_g[g] = max(sA_g[g], math.ceil(max(eA, 1) / 128))
            sB_g[g] = max(sB_g[g], math.ceil(max(eB, 1) / 128))
    assert sA_g.max() <= cfg.smax and sB_g.max() <= cfg.smax

    # global chunk schedule: consecutive windows while sums fit
    chunks = []
    cur, ca, cb = [], 0, 0
    for g in range(gtot):
        if cur and (ca + sA_g[g] > cfg.smax or cb + sB_g[g] > cfg.smax):
            chunks.append(cur)
            cur, ca, cb = [], 0, 0
        cur.append(g)
        ca += sA_g[g]
        cb += sB_g[g]
    if cur:
        chunks.append(cur)

    sched = []
    offA = offB = offD = 0
    for ck in chunks:
        sAtot = int(sum(sA_g[g] for g in ck))
        sBtot = int(sum(sB_g[g] for g in ck))
        wins = []
        pa, pb = 0, 0
        for g in ck:
            wins.append({"g": int(g), "sA": int(sA_g[g]), "sB": int(sB_g[g]),
                         "colA": pa, "colB": sAtot + pb})
            pa += int(sA_g[g])
            pb += int(sB_g[g])
        sched.append({"sA": sAtot, "sB": sBtot, "offA": offA, "offB": offB,
                      "offD": offD, "wins": wins})
        offA += sAtot * 128 // 16
        offB += sBtot * 128 // 16
        offD += sAtot + sBtot
    totA16, totB16, totD = offA, offB, offD

    # per-core gather index / dstl arrays in the global layout
    plans = []
    for c in range(cfg.ncores):
        lo = np.searchsorted(dst_g, bounds[c])
        hi = np.searchsorted(dst_g, bounds[c + 1])
        s = src_n[lo:hi]
        d = dst_g[lo:hi]
        sa = srcA[order][lo:hi] if False else (s < bbase)
        nstart = lo + np.searchsorted(d, np.arange(bounds[c], bounds[c + 1] + 1))

        idxA = np.zeros((128, totA16), np.int16)
        idxB = np.zeros((128, totB16), np.int16)
        dstl = np.full((128, totD), -1.0, np.float16)
        for ck in sched:
            nA, nB = ck["sA"] * 128, ck["sB"] * 128
            arrA = np.zeros(nA, np.int64)
            arrB = np.zeros(nB, np.int64)
            dl = np.full((128, ck["sA"] + ck["sB"]), -1.0, np.float16)
            for w in ck["wins"]:
                n0 = bounds[c] + w["g"] * W
                n1 = min(bounds[c] + (w["g"] + 1) * W, bounds[c + 1])
                if n0 >= n1:
                    continue
                e0 = nstart[n0 - bounds[c]] - lo
                e1 = nstart[n1 - bounds[c]] - lo
                sl = s[e0:e1]
                dloc = (d[e0:e1] - n0).astype(np.int64)
                m = sa[e0:e1]
                iA, dA = sl[m], dloc[m]
                iB, dB = sl[~m] - bbase, dloc[~m]
                assert len(iA) <= w["sA"] * 128 and len(iB) <= w["sB"] * 128
                arrA[w["colA"] * 128: w["colA"] * 128 + len(iA)] = iA
                arrB[(w["colB"] - ck["sA"]) * 128:
                     (w["colB"] - ck["sA"]) * 128 + len(iB)] = iB
                ii = np.arange(len(iA))
                dl[ii % 128, w["colA"] + ii // 128] = dA
                ii = np.arange(len(iB))
                dl[ii % 128, w["colB"] + ii // 128] = dB
            for arr, n, out, off in ((arrA, nA, idxA, ck["offA"]),
                                     (arrB, nB, idxB, ck["offB"])):
                t = np.zeros((16, n // 16), np.int16)
                t[np.arange(n) % 16, np.arange(n) // 16] = arr.astype(np.int16)
                out[:, off:off + n // 16] = np.tile(t, (8, 1))
            dstl[:, ck["offD"]:ck["offD"] + ck["sA"] + ck["sB"]] = dl

        batchloc = np.full((W, gtot), -1.0, np.float16)
        for g in range(gtot):
            n0 = bounds[c] + g * W
            n1 = min(bounds[c] + (g + 1) * W, bounds[c + 1])
            if n0 < n1:
                batchloc[:n1 - n0, g] = batch[n0:n1].astype(np.float16)

        plans.append({"idxA": idxA, "idxB": idxB, "dstl": dstl,
                      "batchloc": batchloc, "bounds": (bounds[c], bounds[c + 1])})

    meta = {"gtot": gtot, "ownp": ownp, "bbase": bbase, "sched": sched,
            "totA16": totA16, "totB16": totB16, "totD": totD}
    return plans, meta


# ----------------------------------------------------------------------------
# device program (shared across all 8 cores)
# ----------------------------------------------------------------------------

def build_nc(cfg: Cfg, meta, debug=False):
    import concourse.bass as bass  # noqa: F401
    import concourse.mybir as mybir
    import concourse.tile as tile
    from concourse import bacc

    fp16 = mybir.dt.float16
    fp8 = mybir.dt.float8e4
    f32 = mybir.dt.float32
    i16 = mybir.dt.int16
    u8 = mybir.dt.uint8
    AL = mybir.AluOpType
    AF = mybir.ActivationFunctionType
    AX = mybir.AxisListType

    gtot, OWNP, bbase = meta["gtot"], meta["ownp"], meta["bbase"]
    sched = meta["sched"]
    W = cfg.win
    couts = [cfg.hid_c, cfg.hid_c, cfg.out_c]
    cins = [cfg.in_c, cfg.hid_c, cfg.hid_c]
    rg = [list(range(cfg.ncores))]
    TROWS = cfg.ncores * OWNP
    ntiles = OWNP // 128

    nc = bacc.Bacc("TRN2", target_bir_lowering=False, debug=debug)

    xT = nc.dram_tensor("xT", [cfg.in_c, OWNP], fp16, kind="ExternalInput")
    Wc, Bb = [], []
    for l in range(3):
        Wc.append(nc.dram_tensor(f"wcat{l}", [cins[l], couts[l] + 2], fp16,
                                 kind="ExternalInput"))
        Bb.append(nc.dram_tensor(f"biasbc{l}", [W, couts[l]], f32,
                                 kind="ExternalInput"))
    iota_d = nc.dram_tensor("iota", [128, W], fp16, kind="ExternalInput")
    onesr_d = nc.dram_tensor("onesrow", [1, 128], fp16, kind="ExternalInput")
    idxA_d = nc.dram_tensor("idxA", [128, meta["totA16"]], i16, kind="ExternalInput")
    idxB_d = nc.dram_tensor("idxB", [128, meta["totB16"]], i16, kind="ExternalInput")
    dstl_d = nc.dram_tensor("dstl", [128, meta["totD"]], fp16, kind="ExternalInput")
    bloc_d = nc.dram_tensor("batchloc", [W, gtot], fp16, kind="ExternalInput")
    out_ext = nc.dram_tensor("out", [cfg.n_graphs, cfg.out_c], f32,
                             kind="ExternalOutput")

    ownt, tbl, adst_d = [], [], []
    for l in range(3):
        ownt.append(nc.dram_tensor(f"ownt{l}", [OWNP, 256], u8))
        tbl.append(nc.dram_tensor(f"tbl{l}", [TROWS, 256], u8, addr_space="Shared"))
        adst_d.append(nc.dram_tensor(f"adst{l}", [OWNP, 1], fp16))
    hb = {l: nc.dram_tensor(f"hb{l}", [OWNP, cfg.hid_c], fp16) for l in (1, 2)}
    pool_l = nc.dram_tensor("pool_local", [cfg.n_graphs, cfg.out_c], f32)
    pool_s = nc.dram_tensor("pool_shared", [cfg.n_graphs, cfg.out_c], f32,
                            addr_space="Shared")

    import contextlib
    with tile.TileContext(nc) as tc, contextlib.ExitStack() as ctx:
        cpool = ctx.enter_context(tc.tile_pool(name="consts", bufs=1))
        apool = ctx.enter_context(tc.tile_pool(name="phasea", bufs=3))
        gpool = ctx.enter_context(tc.tile_pool(name="gather", bufs=2))
        wpool = ctx.enter_context(tc.tile_pool(name="work", bufs=3))
        spool = ctx.enter_context(tc.tile_pool(name="sprime", bufs=2))
        pspool = ctx.enter_context(tc.tile_pool(name="ps", bufs=2, space="PSUM"))
        psw = ctx.enter_context(tc.tile_pool(name="psw", bufs=3, space="PSUM"))
        psacc = ctx.enter_context(tc.tile_pool(name="psacc", bufs=1, space="PSUM"))

        from concourse import library_config
        nc.gpsimd.load_library(library_config.mlp)

        # ---- constants ----
        iota = cpool.tile([128, W], fp16, tag="iota")
        nc.sync.dma_start(iota[:], iota_d[:, :])
        onesr = cpool.tile([1, 128], fp16, tag="onesr")
        nc.sync.dma_start(onesr[:], onesr_d[:, :])
        wcat_sb, bias_sb = [], []
        for l in range(3):
            t = cpool.tile([cins[l], couts[l] + 2], fp16, tag=f"wc{l}")
            nc.sync.dma_start(t[:], Wc[l][:, :])
            wcat_sb.append(t)
            t = cpool.tile([W, couts[l]], f32, tag=f"bb{l}")
            nc.sync.dma_start(t[:], Bb[l][:, :])
            bias_sb.append(t)
        bloc_sb = cpool.tile([W, gtot], fp16, tag="bloc")
        nc.sync.dma_start(bloc_sb[:], bloc_d[:, :])

        zero1 = cpool.tile([128, 1], f32, tag="zero1")
        nc.vector.memset(zero1[:], 0.0)

        psum_pool_acc = psacc.tile([cfg.n_graphs, cfg.out_c], f32, tag="pool")

        for l in range(3):
            cout, cin = couts[l], cins[l]
            use8 = l < 2
            # ======== phase A: own table shard ========
            for t in range(ntiles):
                lhsT = apool.tile([cin, 128], fp16, tag="lhsT")
                if l == 0:
                    nc.sync.dma_start(lhsT[:], xT[:, t * 128:(t + 1) * 128])
                else:
                    nc.sync.dma_start_transpose(
                        lhsT[:], hb[l][t * 128:(t + 1) * 128, 0:cin])
                psA = pspool.tile([128, cout + 2], f32, tag="pa", space="PSUM")
                nc.tensor.matmul(out=psA[:], lhsT=lhsT[:], rhs=wcat_sb[l][:],
                                 start=True, stop=True)
                row = apool.tile([128, 256], u8, tag="row")
                if use8:
                    nc.scalar.activation(out=row[:, 0:128].bitcast(fp8),
                                         in_=psA[:, 0:cout], func=AF.Copy,
                                         scale=cfg.h8scale)
                    nc.vector.memset(row[:, 128:130].bitcast(fp8), 1.0)
                else:
                    nc.scalar.activation(out=row[:, 0:128].bitcast(fp16)[:, 0:cout],
                                         in_=psA[:, 0:cout], func=AF.Copy)
                    nc.vector.memset(row[:, 128:130].bitcast(fp16), 1.0)
                nc.vector.tensor_copy(out=row[:, 130:134].bitcast(fp16),
                                      in_=psA[:, cout:cout + 2])
                nc.sync.dma_start(ownt[l][t * 128:(t + 1) * 128, :], row[:])
                adt = apool.tile([128, 1], fp16, tag="adt")
                nc.vector.tensor_copy(out=adt[:], in_=psA[:, cout + 1:cout + 2])
                nc.sync.dma_start(adst_d[l][t * 128:(t + 1) * 128, :], adt[:])

            # ======== allgather ========
            nc.gpsimd.collective_compute(
                "AllGather", AL.bypass, replica_groups=rg,
                ins=[ownt[l][:, :]], outs=[tbl[l][:, :]])

            # ======== phase B: edge processing ========
            for ck in sched:
                sA, sB = ck["sA"], ck["sB"]
                nA, nB = sA * 128, sB * 128
                ixa = gpool.tile([128, sA * 8], i16, tag="ixa")
                nc.sync.dma_start(ixa[:], idxA_d[:, ck["offA"]:ck["offA"] + sA * 8])
                ixb = gpool.tile([128, sB * 8], i16, tag="ixb")
                nc.sync.dma_start(ixb[:], idxB_d[:, ck["offB"]:ck["offB"] + sB * 8])
                dsl = gpool.tile([128, sA + sB], fp16, tag="dsl")
                nc.sync.dma_start(dsl[:], dstl_d[:, ck["offD"]:ck["offD"] + sA + sB])
                gA = gpool.tile([128, sA, 256], u8, tag="gA")
                nc.gpsimd.dma_gather(gA[:], tbl[l][:, :], ixa[:], nA, nA, 256,
                                     single_packet=cfg.single_packet)
                gB = gpool.tile([128, sB, 256], u8, tag="gB")
                nc.gpsimd.dma_gather(gB[:], tbl[l][bbase:, :], ixb[:], nB, nB, 256,
                                     single_packet=cfg.single_packet)

                # adst of the chunk's own window nodes, broadcast to all
                # 128 partitions via K=1 matmuls (<=512 psum cols per mm)
                nwin = len(ck["wins"])
                g0 = ck["wins"][0]["g"]
                nwcols = nwin * W
                awr = wpool.tile([1, nwcols], fp16, tag="awr")
                nc.sync.dma_start(
                    awr[:], adst_d[l][g0 * W:g0 * W + nwcols, :]
                    .rearrange("n x -> x n"))
                awbc = wpool.tile([128, nwcols], fp16, tag="awbc")
                for c0 in range(0, nwcols, 512):
                    c1 = min(c0 + 512, nwcols)
                    psaw = pspool.tile([128, c1 - c0], f32, tag="psaw",
                                       space="PSUM")
                    nc.tensor.matmul(out=psaw[:], lhsT=onesr[:],
                                     rhs=awr[:, c0:c1], start=True, stop=True)
                    nc.scalar.activation(out=awbc[:, c0:c1], in_=psaw[:],
                                         func=AF.Copy)

                for iw, w in enumerate(ck["wins"]):
                    g = w["g"]
                    nsub = w["sA"] + w["sB"]
                    # S = (iota == dstl) one-hot, per region
                    sp = spool.tile([128, nsub, W], fp16, tag="sp")
                    for (c0, nS, o0) in ((w["colA"], w["sA"], 0),
                                         (w["colB"], w["sB"], w["sA"])):
                        nc.vector.tensor_tensor(
                            out=sp[:, o0:o0 + nS, :],
                            in0=iota[:][:, None, :].to_broadcast([128, nS, W]),
                            in1=dsl[:, c0:c0 + nS][:, :, None].to_broadcast(
                                [128, nS, W]),
                            op=AL.is_equal)
                    # adst per slot = sum_w S * adst_win
                    tmpa = spool.tile([128, nsub, W], fp16, tag="tmpa")
                    nc.vector.tensor_tensor(
                        out=tmpa[:],
                        in0=sp[:],
                        in1=awbc[:, iw * W:(iw + 1) * W][:, None, :]
                        .to_broadcast([128, nsub, W]),
                        op=AL.mult)
                    z = wpool.tile([128, nsub], f32, tag="z")
                    nc.vector.tensor_reduce(out=z[:], in_=tmpa[:],
                                            axis=AX.X, op=AL.add)
                    # z += asrc (from gathered rows, fp16 elem 65)
                    for (gbuf, jj0, nS, o0) in (
                            (gA, w["colA"], w["sA"], 0),
                            (gB, w["colB"] - sA, w["sB"], w["sA"])):
                        nc.vector.tensor_tensor(
                            out=z[:, o0:o0 + nS, None],
                            in0=z[:, o0:o0 + nS, None],
                            in1=gbuf[:].bitcast(fp16)[:, jj0:jj0 + nS, 65:66],
                            op=AL.add)
                    zl = wpool.tile([128, nsub], f32, tag="zl")
                    nc.vector.scalar_tensor_tensor(
                        out=zl[:], in0=z[:], scalar=cfg.neg_slope, in1=z[:],
                        op0=AL.mult, op1=AL.max)
                    zc = wpool.tile([128, nsub], f32, tag="zc")
                    nc.vector.tensor_scalar(out=zc[:], in0=zl[:],
                                            scalar1=cfg.zclamp,
                                            scalar2=-cfg.zshift,
                                            op0=AL.min, op1=AL.add)
                    wv = wpool.tile([128, nsub], fp16, tag="wv")
                    nc.scalar.activation(out=wv[:], in_=zc[:], func=AF.Exp,
                                         bias=zero1[:], scale=1.0)
                    # S' = S * wv
                    nc.vector.tensor_tensor(
                        out=sp[:], in0=sp[:],
                        in1=wv[:][:, :, None].to_broadcast([128, nsub, W]),
                        op=AL.mult)
                    # scatter matmuls: psO[w, :] += S'[:,j,:].T @ [h|1]
                    psO = psw.tile([W, cout + 1], f32, tag="psO", space="PSUM")
                    nsub = w["sA"] + w["sB"]
                    for j in range(nsub):
                        gbuf, jj = (gA, w["colA"] + j) if j < w["sA"] else \
                                   (gB, w["colB"] - sA + (j - w["sA"]))
                        if use8:
                            rhs = gbuf[:].bitcast(fp8)[:, jj, 0:cout + 1]
                        else:
                            rhs = gbuf[:].bitcast(fp16)[:, jj, 0:cout + 1]
                        nc.tensor.matmul(out=psO[:], lhsT=sp[:, j, :], rhs=rhs,
                                         start=(j == 0), stop=(j == nsub - 1))
                    # out = num/den + bias
                    den = wpool.tile([W, 1], f32, tag="den")
                    scl = cfg.h8scale if use8 else 1.0
                    nc.vector.tensor_scalar(out=den[:], in0=psO[:, cout:cout + 1],
                                            scalar1=scl, scalar2=1e-30,
                                            op0=AL.mult, op1=AL.add)
                    rec = wpool.tile([W, 1], f32, tag="rec")
                    nc.vector.reciprocal(out=rec[:], in_=den[:])
                    orow = wpool.tile([W, cout], f32, tag="orow")
                    nc.vector.scalar_tensor_tensor(
                        out=orow[:], in0=psO[:, 0:cout], scalar=rec[:],
                        in1=bias_sb[l][:], op0=AL.mult, op1=AL.add)
                    oh = wpool.tile([W, cout], fp16, tag="oh")
                    if l < 2:
                        nc.scalar.activation(out=oh[:], in_=orow[:], func=AF.Relu)
                        nc.sync.dma_start(hb[l + 1][g * W:(g + 1) * W, 0:cout],
                                          oh[:])
                    else:
                        nc.vector.tensor_copy(out=oh[:], in_=orow[:])
                        bsel = wpool.tile([W, cfg.n_graphs], fp16, tag="bsel")
                        nc.vector.tensor_tensor(
                            out=bsel[:], in0=iota[0:W, 0:cfg.n_graphs],
                            in1=bloc_sb[:, g:g + 1].to_broadcast(
                                [W, cfg.n_graphs]),
                            op=AL.is_equal)
                        nc.tensor.matmul(out=psum_pool_acc[:], lhsT=bsel[:],
                                         rhs=oh[:], start=(g == 0),
                                         stop=(g == gtot - 1))

        # ---- pool -> allreduce -> out ----
        pooled = cpool.tile([cfg.n_graphs, cfg.out_c], f32, tag="pooled")
        nc.vector.tensor_copy(out=pooled[:], in_=psum_pool_acc[:])
        nc.sync.dma_start(pool_l[:, :], pooled[:])
        nc.gpsimd.collective_compute(
            "AllReduce", AL.add, replica_groups=rg,
            ins=[pool_l[:, :]], outs=[pool_s[:, :]])
        nc.sync.dma_start(out_ext[:, :], pool_s[:, :])

    nc.compile()
    return nc


# ----------------------------------------------------------------------------
# host wrapper
# ----------------------------------------------------------------------------

def make_inputs(inputs, plans, meta, cfg: Cfg):
    x = np.asarray(inputs["x"], np.float32)
    ownp, gtot = meta["ownp"], meta["gtot"]
    iota = np.tile(np.arange(cfg.win, dtype=np.float16), (128, 1))
    wcats, biasbcs = [], []
    for l in range(3):
        Wl = np.asarray(inputs[f"W{l}"], np.float32)
        asl = np.asarray(inputs[f"as{l}"], np.float32)
        adl = np.asarray(inputs[f"ad{l}"], np.float32)
        bl = np.asarray(inputs[f"b{l}"], np.float32)
        wcats.append(np.concatenate(
            [Wl, (Wl @ asl)[:, None], (Wl @ adl)[:, None]],
            axis=1).astype(np.float16))
        biasbcs.append(np.tile(bl[None, :], (cfg.win, 1)).astype(np.float32))
    in_maps = []
    for c in range(cfg.ncores):
        p = plans[c]
        lo, hi = p["bounds"]
        xpad = np.zeros((ownp, cfg.in_c), np.float32)
        xpad[:hi - lo] = x[lo:hi]
        m = {"xT": np.ascontiguousarray(xpad.T).astype(np.float16),
             "iota": iota, "onesrow": np.ones((1, 128), np.float16),
             "idxA": p["idxA"], "idxB": p["idxB"], "dstl": p["dstl"],
             "batchloc": p["batchloc"]}
        for l in range(3):
            m[f"wcat{l}"] = wcats[l]
            m[f"biasbc{l}"] = biasbcs[l]
        in_maps.append(m)
    return in_maps


def kernel(**inputs) -> np.ndarray:
    cfg = FULL
    edge_index = np.asarray(inputs["edge_index"])
    batch = np.asarray(inputs["batch"])
    plans, meta = build_plan(edge_index, batch, cfg)
    in_maps = make_inputs(inputs, plans, meta, cfg)
    nc = build_nc(cfg, meta, debug=False)
    from concourse import bass_utils
    res = bass_utils.run_bass_kernel_spmd(nc, in_maps, core_ids=list(range(cfg.ncores)))
    return np.asarray(res.results[0]["out"], np.float32)
